# revision 15
# baseline (speedup 1.0000x reference)
"""TRN2 Bass kernel: fused LSTM cell (nn_CustomLSTMCell), 8-core tensor-parallel.

Strategy
--------
gates = x @ W_ih.T + b_ih + h_prev @ W_hh.T + b_hh  is computed as ONE GEMM
with contraction K = I + H = 4096 over xh = [x | h_prev] and W = [W_ih | W_hh].

The 4H gate dimension is tensor-parallel sharded across the 8 cores: core c
owns h-columns [c*256, (c+1)*256) of every gate (i, f, g, o).  Each core
computes gatesT [1024, 2048] = Wc @ xh.T with gate rows on partitions, so the
per-gate bias is a native per-partition scalar in scalar.activation, which
also applies sigmoid/tanh while evicting PSUM -> SBUF.  The LSTM cell update
(new_C = f*C + i*g, new_h = o*tanh(new_C)) runs on the vector engine, fully
overlapped with the tensor engine.  No collectives: output slices are
disjoint and gathered on the host.

Matmul operands are cast to fp16 on the host (halves DMA traffic, 4x PE rate
vs fp32, ~8x more mantissa than bf16); accumulation stays fp32 in PSUM and
the epilogue is fp32.

Schedule: hb (128-row half of the per-core gate slice) is the OUTER loop and
the batch tile n the inner loop, so only the lo half of W (4MB) is needed in
the front window.  The first TWO batch tiles are fused into one double-wide
group (chunk-major over both 512-col halves, 8 PSUM banks) so W-lo and the
first two xh tiles are consumed as per-chunk triples over 54.6us (~225GB/s,
inside the ~300GB/s the queues sustain).  Phase hi re-reads the xh tiles.

DMA dispatch instructions cost ~0.6us of engine sequencer time EACH, so the
dispatch count is the scarce resource (a per-chunk re-read schedule needed
~290 dispatches and starved the PE for 21us waiting on un-dispatched loads).
Only the front window uses per-chunk DMAs (pacing matters there); every
prefetched xh set, W-hi, and C ride ONE large DMA each -- they land a full
group ahead, so whole-set completion granularity costs nothing.

A short PE warm-up (6 dummy matmuls, ~2.6us) bridges the gap until chunk 0
lands and starts the HAM clock-gate warm-up early.  All output stores go on
the two HWDGE queues (sync/scalar); the final group pre-stores f/i/g/cn
during the o-gate matmul block so only o-sigmoid -> h-mul -> 2 half-stores
remain after the last MM.
"""

import numpy as np

B = 2048           # batch
I_DIM = 2048       # input features
H = 2048           # hidden
NCORES = 8
S = H // NCORES    # 256: per-core h-slice (per gate)
M_PER_CORE = 4 * S # 1024 gate rows per core
K = I_DIM + H      # 4096 fused contraction dim
P = 128
KC = K // P        # 32 contraction chunks
NT = B // 512      # 4 batch tiles of 512

_BF16 = np.float16

_CACHE = {}


def _build_program():
    from contextlib import ExitStack

    import concourse.mybir as mybir
    import concourse.tile as tile
    from concourse import bacc

    f32 = mybir.dt.float32
    bf16 = mybir.dt.float16
    AF = mybir.ActivationFunctionType

    nc = bacc.Bacc("TRN2", target_bir_lowering=False, debug=False)

    w_t = nc.dram_tensor("w_t", [K, M_PER_CORE], bf16, kind="ExternalInput").ap()
    xh_t = nc.dram_tensor("xh_t", [K, B], bf16, kind="ExternalInput").ap()
    # bias pre-shaped [128, 8] on the host: one clean 2D DMA (32B/partition)
    bias_d = nc.dram_tensor("bias", [P, 8], f32, kind="ExternalInput").ap()
    c_t = nc.dram_tensor("c_t", [S, B], f32, kind="ExternalInput").ap()
    outs = {
        name: nc.dram_tensor(name, [S, B], f32, kind="ExternalOutput").ap()
        for name in ("h_t", "cn_t", "f_t", "i_t", "g_t", "o_t")
    }

    # DRAM views with the 128-partition dim innermost on rows.
    w_r = w_t.rearrange("(a p) m -> p a m", p=P)        # [128, 32, 1024]
    xh_r = xh_t.rearrange("(a p) n -> p a n", p=P)      # [128, 32, 2048]
    c_r = c_t.rearrange("(a p) n -> p a n", p=P)        # [128, 2, 2048]

    HB = S // P  # 2 h-blocks of 128 per core
    # gate order within the per-core M dim: m-tile = hb*4 + gate (i,f,g,o)
    ACT_FN = [AF.Sigmoid, AF.Sigmoid, AF.Tanh, AF.Sigmoid]

    # group schedule: hb outer, n inner; groups 0+1 are fused (see docstring)
    SCHED = [(0, 0), (0, 1), (0, 2), (0, 3), (1, 0), (1, 1), (1, 2), (1, 3)]

    with tile.TileContext(nc) as tc, ExitStack() as ctx:
        w_pool = ctx.enter_context(tc.tile_pool(name="w", bufs=1))
        xh_pool = ctx.enter_context(tc.tile_pool(name="xh", bufs=3))
        c_pool = ctx.enter_context(tc.tile_pool(name="c", bufs=1))
        b_pool = ctx.enter_context(tc.tile_pool(name="b", bufs=1))
        psum_pool = ctx.enter_context(tc.tile_pool(name="ps", bufs=2, space="PSUM"))
        act_pool = ctx.enter_context(tc.tile_pool(name="act", bufs=2))

        in_eng = [nc.sync, nc.gpsimd]
        _in_rr = [0]

        def in_dma(dst, src):
            in_eng[_in_rr[0] % 2].dma_start(dst, src)
            _in_rr[0] += 1

        in3_eng = [nc.sync, nc.gpsimd, nc.scalar]
        _in3_rr = [0]

        def in3_dma(dst, src):
            in3_eng[_in3_rr[0] % 3].dma_start(dst, src)
            _in3_rr[0] += 1

        # Mid-kernel output stores ride the Scalar HWDGE queue, naturally
        # interleaved behind the activations that produce them.
        def out_dma(dst, src):
            nc.scalar.dma_start(dst, src)

        # Load bias first: tiny, and the first epilogue needs it.
        bias_all = b_pool.tile([P, 4 * HB], f32)
        nc.sync.dma_start(bias_all[:], bias_d[:, :])

        # W-lo: per-chunk tiles -- 128KB DMAs produce 1KB-per-partition
        # packets, measured FASTER per byte than the 2KB packets that
        # chunk-pair DMAs generate.  W-hi: two 2MB halves (8KB+ packets,
        # also fast), emitted a group apart so each lands with slack
        # before phase hi consumes it.
        w_lo = [w_pool.tile([P, 4 * P], bf16, name=f"wl{k}") for k in range(KC)]
        w_hi = [
            w_pool.tile([P, KC // 2, 4 * P], bf16, name=f"wh{h}")
            for h in range(2)
        ]

        def w_slice(hb, k, g):
            if hb == 0:
                return w_lo[k][:, g * P : (g + 1) * P]
            return w_hi[k // 16][:, k % 16, g * P : (g + 1) * P]

        # All xh sets are uniform 3D tiles (same pool slot footprint).
        # Sets 0-2 (front window, pacing matters) are written by 32
        # per-chunk DMAs into [:, k, :] slices -- Tile tracks sub-tile
        # ranges, so each matmul waits only on its own chunk's DMA.
        # Sets 3-7 are prefetched a full group ahead with ONE 4MB DMA each
        # (dispatch instructions cost ~0.6us of engine time apiece).
        def alloc_xh():
            return xh_pool.tile([P, KC, 512], bf16, name="xhs")

        xh_sets = {0: alloc_xh(), 1: alloc_xh(), 2: alloc_xh()}

        def xh_chunk(i, k):
            return xh_sets[i][:, k, :]

        # Preamble.  Chunk 0 of the (w, xh0, xh1) triple goes out first on
        # the HWDGE queues so the first matmul can start ASAP; the rest
        # interleaves per-chunk triples over all three queues in the fused
        # group's consumption order, then C-lo, then set 2 (consumed
        # chunk-paced by group 2 right after the fused group).
        # The gpsimd (SWDGE) queue only sustains ~65GB/s on this pattern,
        # so NOTHING deadline-critical rides it: the fused group's three
        # streams live on the two HWDGE queues (w on sync, xh0 on scalar,
        # xh1 alternating), in consumption order.  gpsimd gets half of
        # set 2, whose deadline is a whole group away.
        nc.sync.dma_start(w_lo[0][:], w_r[:, 0, 0 : 4 * P])
        nc.sync.dma_start(xh_sets[0][:, 0, :], xh_r[:, 0, 0:512])
        nc.scalar.dma_start(xh_sets[1][:, 0, :], xh_r[:, 0, 512:1024])
        for k in range(1, KC):
            nc.sync.dma_start(w_lo[k][:], w_r[:, k, 0 : 4 * P])
            nc.scalar.dma_start(xh_sets[0][:, k, :], xh_r[:, k, 0:512])
            eng = nc.sync if k % 2 == 0 else nc.scalar
            eng.dma_start(xh_sets[1][:, k, :], xh_r[:, k, 512:1024])
        c_tiles = {0: c_pool.tile([P, B], f32, name="c")}
        nc.scalar.dma_start(c_tiles[0][:], c_r[:, 0, :])
        for k in range(KC):
            eng = nc.gpsimd if k % 2 == 0 else (nc.sync if k % 4 == 1 else nc.scalar)
            eng.dma_start(xh_sets[2][:, k, :], xh_r[:, k, 1024:1536])

        # PE warm-up: a few matmuls on dummy data bridge the gap until
        # chunk 0 lands (~10.5us) and start the HAM warm-up window early.
        dummy = b_pool.tile([P, 512], bf16)
        nc.vector.memset(dummy[:], 0.0)
        warm_ps = psum_pool.tile([P, 512], f32, name="ps0")
        NWARM = 6
        for i in range(NWARM):
            nc.tensor.matmul(
                warm_ps[:], dummy[:, 0:P], dummy[:],
                start=(i == 0), stop=(i == NWARM - 1),
            )

        def epilogue(hb, ns, ps, final):
            # activations (+bias) evict PSUM, then the cell update.  For
            # the final group: f,i,g evict first and the cell-state chain
            # plus the f/i/g/cn stores complete during the o matmul block.
            def gate_act(g):
                m = hb * 4 + g
                a = act_pool.tile([P, 512], f32, name=f"a{g}")
                nc.scalar.activation(
                    a[:], ps[g][:], ACT_FN[g], bias=bias_all[:, m : m + 1]
                )
                return a

            acts = [None] * 4
            for g in (1, 0, 2) if final else (0, 1, 2, 3):
                acts[g] = gate_act(g)
            fc = act_pool.tile([P, 512], f32, name="fc")
            nc.vector.tensor_mul(fc[:], acts[1][:], c_tiles[hb][:, ns])
            ig = act_pool.tile([P, 512], f32, name="ig")
            nc.vector.tensor_mul(ig[:], acts[0][:], acts[2][:])
            cn = act_pool.tile([P, 512], f32, name="cn")
            nc.vector.tensor_add(cn[:], ig[:], fc[:])
            th = act_pool.tile([P, 512], f32, name="th")
            nc.scalar.activation(th[:], cn[:], AF.Tanh)

            rs = slice(hb * P, (hb + 1) * P)
            if final:
                # pre-store everything ready while o's matmuls run
                nc.sync.dma_start(outs["f_t"][rs, ns], acts[1][:])
                nc.sync.dma_start(outs["i_t"][rs, ns], acts[0][:])
                nc.scalar.dma_start(outs["g_t"][rs, ns], acts[2][:])
                nc.sync.dma_start(outs["cn_t"][rs, ns], cn[:])
                acts[3] = gate_act(3)
            hn = act_pool.tile([P, 512], f32, name="hn")
            nc.vector.tensor_mul(hn[:], acts[3][:], th[:])

            if final:
                # split h/o across both HWDGE queues for parallel receipt
                lo = slice(ns.start, ns.start + 256)
                hi = slice(ns.start + 256, ns.stop)
                nc.scalar.dma_start(outs["o_t"][rs, lo], acts[3][:, 0:256])
                nc.sync.dma_start(outs["o_t"][rs, hi], acts[3][:, 256:512])
                nc.sync.dma_start(outs["h_t"][rs, lo], hn[:, 0:256])
                nc.scalar.dma_start(outs["h_t"][rs, hi], hn[:, 256:512])
            else:
                pairs = (
                    ("f_t", acts[1]),
                    ("i_t", acts[0]),
                    ("g_t", acts[2]),
                    ("o_t", acts[3]),
                    ("cn_t", cn),
                    ("h_t", hn),
                )
                for name, t in pairs:
                    out_dma(outs[name][rs, ns], t[:])

        # ---- fused double-wide first group: batch cols [0:1024], hb=0 ----
        # chunk-major over both halves: the (w, xh0, xh1) triple of chunk k
        # is consumed at ~k*1.7us, matching the preamble queue order.
        ps01 = [
            [psum_pool.tile([P, 512], f32, name=f"ps{g}") for g in range(4)]
            for _ in range(2)
        ]
        for k in range(KC):
            for half in range(2):
                for g in range(4):
                    nc.tensor.matmul(
                        ps01[half][g][:],
                        w_slice(0, k, g),
                        xh_chunk(half, k),
                        start=(k == 0),
                        stop=(k == KC - 1),
                    )
        # set 3 (one DMA): pool slot is set 0's, so it waits for the fused
        # group's reads and lands during group 2, consumed in group 3.
        xh_sets[3] = alloc_xh()
        nc.sync.dma_start(xh_sets[3][:], xh_r[:, :, 1536:2048])
        # W-hi half 0 after the fused group: gpsimd queues it behind
        # nothing deadline-critical; lands during group 2/3.
        for half in range(2):
            epilogue(0, slice(half * 512, (half + 1) * 512), ps01[half], False)

        # ---- groups 2..7 ----
        for i in range(2, len(SCHED)):
            final = i == len(SCHED) - 1
            hb, n = SCHED[i]
            ns = slice(n * 512, (n + 1) * 512)

            ps = [
                psum_pool.tile([P, 512], f32, name=f"ps{g}") for g in range(4)
            ]
            if final:
                # gate-major (f,i,g,o): each gate's PSUM closes early so the
                # epilogue chain after the very last matmul is just
                # o-sigmoid -> h-mul -> store.
                for g in (1, 0, 2, 3):
                    for k in range(KC):
                        nc.tensor.matmul(
                            ps[g][:],
                            w_slice(hb, k, g),
                            xh_chunk(i, k),
                            start=(k == 0),
                            stop=(k == KC - 1),
                        )
            else:
                for k in range(KC):
                    for g in range(4):
                        nc.tensor.matmul(
                            ps[g][:],
                            w_slice(hb, k, g),
                            xh_chunk(i, k),
                            start=(k == 0),
                            stop=(k == KC - 1),
                        )

            # Prefetch xh set i+2 as ONE DMA after this group's matmuls:
            # its pool-slot wait self-paces it into the next group's window.
            if i + 2 < len(SCHED):
                xh_sets[i + 2] = alloc_xh()
                nn = SCHED[i + 2][1]
                nc.sync.dma_start(
                    xh_sets[i + 2][:], xh_r[:, :, nn * 512 : (nn + 1) * 512]
                )
            # W-hi halves after groups 2 and 3 (each lands a group before
            # phase hi needs it); C-hi (slot of C-lo) waits for group 3's
            # epilogue and lands mid-group-4, before its epilogue needs it.
            if i == 2:
                nc.sync.dma_start(w_hi[0][:], w_r[:, 0:16, 4 * P : 8 * P])
            elif i == 3:
                nc.sync.dma_start(w_hi[1][:], w_r[:, 16:32, 4 * P : 8 * P])
                c_tiles[1] = c_pool.tile([P, B], f32, name="c")
                nc.scalar.dma_start(c_tiles[1][:], c_r[:, 1, :])

            epilogue(hb, ns, ps, final)

    nc.compile()
    return nc


def _get_program():
    if "nc" not in _CACHE:
        _CACHE["nc"] = _build_program()
    return _CACHE["nc"]


def _gate_row_index(core: int) -> np.ndarray:
    """Global rows of W/b (4H-dim) owned by `core`, in m-tile order."""
    idx = []
    for hb in range(S // P):
        for g in range(4):
            base = g * H + core * S + hb * P
            idx.extend(range(base, base + P))
    return np.asarray(idx)


def kernel(x, h_prev, C_prev, W_ih, b_ih, W_hh, b_hh):
    from concourse.bass_utils import run_bass_kernel_spmd

    nc = _get_program()

    xh_t = np.ascontiguousarray(
        np.concatenate([x, h_prev], axis=1).T
    ).astype(_BF16)  # [4096, 2048], shared by all cores
    bias_full = (b_ih + b_hh).astype(np.float32)

    in_maps = []
    for c in range(NCORES):
        idx = _gate_row_index(c)
        w_cat = np.concatenate([W_ih[idx], W_hh[idx]], axis=1)  # [1024, 4096]
        in_maps.append(
            {
                "w_t": np.ascontiguousarray(w_cat.T).astype(_BF16),
                "xh_t": xh_t,
                "bias": np.ascontiguousarray(bias_full[idx].reshape(8, P).T),
                "c_t": np.ascontiguousarray(C_prev[:, c * S : (c + 1) * S].T),
            }
        )

    _CACHE["last_in_maps"] = in_maps
    res = run_bass_kernel_spmd(nc, in_maps, core_ids=list(range(NCORES)))

    def gather(name):
        t = np.concatenate([res.results[c][name] for c in range(NCORES)], axis=0)
        return np.ascontiguousarray(t.T)  # [B, H]

    return (
        gather("h_t"),
        gather("cn_t"),
        gather("f_t"),
        gather("i_t"),
        gather("g_t"),
        gather("o_t"),
    )


# revision 17
# speedup vs baseline: 1.0276x; 1.0276x over previous
"""TRN2 Bass kernel: fused LSTM cell (nn_CustomLSTMCell), 8-core tensor-parallel.

Strategy
--------
gates = x @ W_ih.T + b_ih + h_prev @ W_hh.T + b_hh  is computed as ONE GEMM
with contraction K = I + H = 4096 over xh = [x | h_prev] and W = [W_ih | W_hh].

The 4H gate dimension is tensor-parallel sharded across the 8 cores: core c
owns h-columns [c*256, (c+1)*256) of every gate (i, f, g, o).  Each core
computes gatesT [1024, 2048] = Wc @ xh.T with gate rows on partitions, so the
per-gate bias is a native per-partition scalar in scalar.activation, which
also applies sigmoid/tanh while evicting PSUM -> SBUF.  The LSTM cell update
(new_C = f*C + i*g, new_h = o*tanh(new_C)) runs on the vector engine, fully
overlapped with the tensor engine.  No collectives: output slices are
disjoint and gathered on the host.

Matmul operands are cast to fp16 on the host (halves DMA traffic, 4x PE rate
vs fp32, ~8x more mantissa than bf16); accumulation stays fp32 in PSUM and
the epilogue is fp32.

Schedule: hb (128-row half of the per-core gate slice) is the OUTER loop and
the batch tile n the inner loop, so only the lo half of W (4MB) is needed in
the front window.  The first TWO batch tiles are fused into one double-wide
group (chunk-major over both 512-col halves, 8 PSUM banks) so W-lo and the
first two xh tiles are consumed as per-chunk triples over 54.6us (~225GB/s,
inside the ~300GB/s the queues sustain).  Phase hi re-reads the xh tiles.

DMA dispatch instructions cost ~0.6us of engine sequencer time EACH, so the
dispatch count is the scarce resource (a per-chunk re-read schedule needed
~290 dispatches and starved the PE for 21us waiting on un-dispatched loads).
Only the front window uses per-chunk DMAs (pacing matters there); every
prefetched xh set, W-hi, and C ride ONE large DMA each -- they land a full
group ahead, so whole-set completion granularity costs nothing.

A short PE warm-up (6 dummy matmuls, ~2.6us) bridges the gap until chunk 0
lands and starts the HAM clock-gate warm-up early.  All output stores go on
the two HWDGE queues (sync/scalar); the final group pre-stores f/i/g/cn
during the o-gate matmul block so only o-sigmoid -> h-mul -> 2 half-stores
remain after the last MM.
"""

import numpy as np

B = 2048           # batch
I_DIM = 2048       # input features
H = 2048           # hidden
NCORES = 8
S = H // NCORES    # 256: per-core h-slice (per gate)
M_PER_CORE = 4 * S # 1024 gate rows per core
K = I_DIM + H      # 4096 fused contraction dim
P = 128
KC = K // P        # 32 contraction chunks
NT = B // 512      # 4 batch tiles of 512

_BF16 = np.float16

_CACHE = {}


def _build_program():
    from contextlib import ExitStack

    import concourse.mybir as mybir
    import concourse.tile as tile
    from concourse import bacc

    f32 = mybir.dt.float32
    bf16 = mybir.dt.float16
    AF = mybir.ActivationFunctionType

    nc = bacc.Bacc("TRN2", target_bir_lowering=False, debug=False)

    w_t = nc.dram_tensor("w_t", [K, M_PER_CORE], bf16, kind="ExternalInput").ap()
    xh_t = nc.dram_tensor("xh_t", [K, B], bf16, kind="ExternalInput").ap()
    # bias pre-shaped [128, 8] on the host: one clean 2D DMA (32B/partition)
    bias_d = nc.dram_tensor("bias", [P, 8], f32, kind="ExternalInput").ap()
    c_t = nc.dram_tensor("c_t", [S, B], f32, kind="ExternalInput").ap()
    outs = {
        name: nc.dram_tensor(name, [S, B], f32, kind="ExternalOutput").ap()
        for name in ("h_t", "cn_t", "f_t", "i_t", "g_t", "o_t")
    }

    # DRAM views with the 128-partition dim innermost on rows.
    w_r = w_t.rearrange("(a p) m -> p a m", p=P)        # [128, 32, 1024]
    xh_r = xh_t.rearrange("(a p) n -> p a n", p=P)      # [128, 32, 2048]
    c_r = c_t.rearrange("(a p) n -> p a n", p=P)        # [128, 2, 2048]

    HB = S // P  # 2 h-blocks of 128 per core
    # gate order within the per-core M dim: m-tile = hb*4 + gate (i,f,g,o)
    ACT_FN = [AF.Sigmoid, AF.Sigmoid, AF.Tanh, AF.Sigmoid]

    # group schedule: hb outer, n inner; groups 0+1 are fused (see docstring)
    SCHED = [(0, 0), (0, 1), (0, 2), (0, 3), (1, 0), (1, 1), (1, 2), (1, 3)]

    with tile.TileContext(nc) as tc, ExitStack() as ctx:
        w_pool = ctx.enter_context(tc.tile_pool(name="w", bufs=1))
        xh_pool = ctx.enter_context(tc.tile_pool(name="xh", bufs=3))
        c_pool = ctx.enter_context(tc.tile_pool(name="c", bufs=1))
        b_pool = ctx.enter_context(tc.tile_pool(name="b", bufs=1))
        psum_pool = ctx.enter_context(tc.tile_pool(name="ps", bufs=2, space="PSUM"))
        act_pool = ctx.enter_context(tc.tile_pool(name="act", bufs=2))

        in_eng = [nc.sync, nc.gpsimd]
        _in_rr = [0]

        def in_dma(dst, src):
            in_eng[_in_rr[0] % 2].dma_start(dst, src)
            _in_rr[0] += 1

        in3_eng = [nc.sync, nc.gpsimd, nc.scalar]
        _in3_rr = [0]

        def in3_dma(dst, src):
            in3_eng[_in3_rr[0] % 3].dma_start(dst, src)
            _in3_rr[0] += 1

        # Mid-kernel output stores ride the Scalar HWDGE queue, naturally
        # interleaved behind the activations that produce them.
        def out_dma(dst, src):
            nc.scalar.dma_start(dst, src)

        bias_all = b_pool.tile([P, 4 * HB], f32)

        # W-lo: per-chunk tiles -- 128KB DMAs produce 1KB-per-partition
        # packets, measured FASTER per byte than the 2KB packets that
        # chunk-pair DMAs generate.  W-hi: two 2MB halves (8KB+ packets,
        # also fast), emitted a group apart so each lands with slack
        # before phase hi consumes it.
        w_lo = [w_pool.tile([P, 4 * P], bf16, name=f"wl{k}") for k in range(KC)]
        w_hi = [
            w_pool.tile([P, KC // 2, 4 * P], bf16, name=f"wh{h}")
            for h in range(2)
        ]

        def w_slice(hb, k, g):
            if hb == 0:
                return w_lo[k][:, g * P : (g + 1) * P]
            return w_hi[k // 16][:, k % 16, g * P : (g + 1) * P]

        # All xh sets are uniform 3D tiles (same pool slot footprint).
        # Sets 0-2 (front window, pacing matters) are written by 32
        # per-chunk DMAs into [:, k, :] slices -- Tile tracks sub-tile
        # ranges, so each matmul waits only on its own chunk's DMA.
        # Sets 3-7 are prefetched a full group ahead with ONE 4MB DMA each
        # (dispatch instructions cost ~0.6us of engine time apiece).
        def alloc_xh():
            return xh_pool.tile([P, KC, 512], bf16, name="xhs")

        xh_sets = {0: alloc_xh(), 1: alloc_xh(), 2: alloc_xh()}

        def xh_chunk(i, k):
            return xh_sets[i][:, k, :]

        # Preamble.  Chunk 0 of the (w, xh0, xh1) triple goes out first on
        # the HWDGE queues so the first matmul can start ASAP; the rest
        # interleaves per-chunk triples over all three queues in the fused
        # group's consumption order, then C-lo, then set 2 (consumed
        # chunk-paced by group 2 right after the fused group).
        # The gpsimd (SWDGE) queue only sustains ~65GB/s on this pattern,
        # so NOTHING deadline-critical rides it: the fused group's three
        # streams live on the two HWDGE queues (w on sync, xh0 on scalar,
        # xh1 alternating), in consumption order.  gpsimd gets half of
        # set 2, whose deadline is a whole group away.
        nc.sync.dma_start(w_lo[0][:], w_r[:, 0, 0 : 4 * P])
        nc.sync.dma_start(xh_sets[0][:, 0, :], xh_r[:, 0, 0:512])
        nc.scalar.dma_start(xh_sets[1][:, 0, :], xh_r[:, 0, 512:1024])
        # bias is tiny and first needed at ~65us; keep it off chunk 0's path
        nc.sync.dma_start(bias_all[:], bias_d[:, :])
        for k in range(1, KC):
            nc.sync.dma_start(w_lo[k][:], w_r[:, k, 0 : 4 * P])
            nc.scalar.dma_start(xh_sets[0][:, k, :], xh_r[:, k, 0:512])
            eng = nc.sync if k % 2 == 0 else nc.scalar
            eng.dma_start(xh_sets[1][:, k, :], xh_r[:, k, 512:1024])
        c_tiles = {0: c_pool.tile([P, B], f32, name="c")}
        nc.scalar.dma_start(c_tiles[0][:], c_r[:, 0, :])
        for k in range(KC):
            eng = nc.gpsimd if k % 2 == 0 else (nc.sync if k % 4 == 1 else nc.scalar)
            eng.dma_start(xh_sets[2][:, k, :], xh_r[:, k, 1024:1536])

        # PE warm-up: a few matmuls on dummy data bridge the gap until
        # chunk 0 lands (~10.5us) and start the HAM warm-up window early.
        dummy = b_pool.tile([P, 512], bf16)
        nc.vector.memset(dummy[:], 0.0)
        warm_ps = psum_pool.tile([P, 512], f32, name="ps0")
        NWARM = 8
        for i in range(NWARM):
            nc.tensor.matmul(
                warm_ps[:], dummy[:, 0:P], dummy[:],
                start=(i == 0), stop=(i == NWARM - 1),
            )

        def epilogue(hb, ns, ps, final):
            # activations (+bias) evict PSUM, then the cell update.  For
            # the final group: f,i,g evict first and the cell-state chain
            # plus the f/i/g/cn stores complete during the o matmul block.
            def gate_act(g):
                m = hb * 4 + g
                a = act_pool.tile([P, 512], f32, name=f"a{g}")
                nc.scalar.activation(
                    a[:], ps[g][:], ACT_FN[g], bias=bias_all[:, m : m + 1]
                )
                return a

            acts = [None] * 4
            for g in (1, 0, 2) if final else (0, 1, 2, 3):
                acts[g] = gate_act(g)
            fc = act_pool.tile([P, 512], f32, name="fc")
            nc.vector.tensor_mul(fc[:], acts[1][:], c_tiles[hb][:, ns])
            ig = act_pool.tile([P, 512], f32, name="ig")
            nc.vector.tensor_mul(ig[:], acts[0][:], acts[2][:])
            cn = act_pool.tile([P, 512], f32, name="cn")
            nc.vector.tensor_add(cn[:], ig[:], fc[:])
            th = act_pool.tile([P, 512], f32, name="th")
            nc.scalar.activation(th[:], cn[:], AF.Tanh)

            rs = slice(hb * P, (hb + 1) * P)
            if final:
                # pre-store everything ready while o's matmuls run
                nc.sync.dma_start(outs["f_t"][rs, ns], acts[1][:])
                nc.sync.dma_start(outs["i_t"][rs, ns], acts[0][:])
                nc.scalar.dma_start(outs["g_t"][rs, ns], acts[2][:])
                nc.sync.dma_start(outs["cn_t"][rs, ns], cn[:])
                acts[3] = gate_act(3)
            hn = act_pool.tile([P, 512], f32, name="hn")
            nc.vector.tensor_mul(hn[:], acts[3][:], th[:])

            if final:
                # split h/o across both HWDGE queues for parallel receipt
                lo = slice(ns.start, ns.start + 256)
                hi = slice(ns.start + 256, ns.stop)
                nc.scalar.dma_start(outs["o_t"][rs, lo], acts[3][:, 0:256])
                nc.sync.dma_start(outs["o_t"][rs, hi], acts[3][:, 256:512])
                nc.sync.dma_start(outs["h_t"][rs, lo], hn[:, 0:256])
                nc.scalar.dma_start(outs["h_t"][rs, hi], hn[:, 256:512])
            else:
                pairs = (
                    ("f_t", acts[1]),
                    ("i_t", acts[0]),
                    ("g_t", acts[2]),
                    ("o_t", acts[3]),
                    ("cn_t", cn),
                    ("h_t", hn),
                )
                for name, t in pairs:
                    out_dma(outs[name][rs, ns], t[:])

        # ---- fused double-wide first group: batch cols [0:1024], hb=0 ----
        # chunk-major over both halves: the (w, xh0, xh1) triple of chunk k
        # is consumed at ~k*1.7us, matching the preamble queue order.
        ps01 = [
            [psum_pool.tile([P, 512], f32, name=f"ps{g}") for g in range(4)]
            for _ in range(2)
        ]
        for k in range(KC):
            for half in range(2):
                for g in range(4):
                    nc.tensor.matmul(
                        ps01[half][g][:],
                        w_slice(0, k, g),
                        xh_chunk(half, k),
                        start=(k == 0),
                        stop=(k == KC - 1),
                    )
        # set 3 (one DMA): pool slot is set 0's, so it waits for the fused
        # group's reads and lands during group 2, consumed in group 3.
        xh_sets[3] = alloc_xh()
        nc.sync.dma_start(xh_sets[3][:], xh_r[:, :, 1536:2048])
        # W-hi half 0 after the fused group: gpsimd queues it behind
        # nothing deadline-critical; lands during group 2/3.
        for half in range(2):
            epilogue(0, slice(half * 512, (half + 1) * 512), ps01[half], False)

        # ---- groups 2..7 ----
        for i in range(2, len(SCHED)):
            final = i == len(SCHED) - 1
            hb, n = SCHED[i]
            ns = slice(n * 512, (n + 1) * 512)

            ps = [
                psum_pool.tile([P, 512], f32, name=f"ps{g}") for g in range(4)
            ]
            if final:
                # gate-major (f,i,g,o): each gate's PSUM closes early so the
                # epilogue chain after the very last matmul is just
                # o-sigmoid -> h-mul -> store.
                for g in (1, 0, 2, 3):
                    for k in range(KC):
                        nc.tensor.matmul(
                            ps[g][:],
                            w_slice(hb, k, g),
                            xh_chunk(i, k),
                            start=(k == 0),
                            stop=(k == KC - 1),
                        )
            else:
                for k in range(KC):
                    for g in range(4):
                        nc.tensor.matmul(
                            ps[g][:],
                            w_slice(hb, k, g),
                            xh_chunk(i, k),
                            start=(k == 0),
                            stop=(k == KC - 1),
                        )

            # Prefetch xh set i+2 as ONE DMA after this group's matmuls:
            # its pool-slot wait self-paces it into the next group's window.
            if i + 2 < len(SCHED):
                xh_sets[i + 2] = alloc_xh()
                nn = SCHED[i + 2][1]
                nc.sync.dma_start(
                    xh_sets[i + 2][:], xh_r[:, :, nn * 512 : (nn + 1) * 512]
                )
            # W-hi halves after groups 2 and 3 (each lands a group before
            # phase hi needs it); C-hi (slot of C-lo) waits for group 3's
            # epilogue and lands mid-group-4, before its epilogue needs it.
            if i == 2:
                nc.sync.dma_start(w_hi[0][:], w_r[:, 0:16, 4 * P : 8 * P])
            elif i == 3:
                nc.sync.dma_start(w_hi[1][:], w_r[:, 16:32, 4 * P : 8 * P])
                c_tiles[1] = c_pool.tile([P, B], f32, name="c")
                nc.scalar.dma_start(c_tiles[1][:], c_r[:, 1, :])

            epilogue(hb, ns, ps, final)

    nc.compile()
    return nc


def _get_program():
    if "nc" not in _CACHE:
        _CACHE["nc"] = _build_program()
    return _CACHE["nc"]


def _gate_row_index(core: int) -> np.ndarray:
    """Global rows of W/b (4H-dim) owned by `core`, in m-tile order."""
    idx = []
    for hb in range(S // P):
        for g in range(4):
            base = g * H + core * S + hb * P
            idx.extend(range(base, base + P))
    return np.asarray(idx)


def kernel(x, h_prev, C_prev, W_ih, b_ih, W_hh, b_hh):
    from concourse.bass_utils import run_bass_kernel_spmd

    nc = _get_program()

    xh_t = np.ascontiguousarray(
        np.concatenate([x, h_prev], axis=1).T
    ).astype(_BF16)  # [4096, 2048], shared by all cores
    bias_full = (b_ih + b_hh).astype(np.float32)

    in_maps = []
    for c in range(NCORES):
        idx = _gate_row_index(c)
        w_cat = np.concatenate([W_ih[idx], W_hh[idx]], axis=1)  # [1024, 4096]
        in_maps.append(
            {
                "w_t": np.ascontiguousarray(w_cat.T).astype(_BF16),
                "xh_t": xh_t,
                "bias": np.ascontiguousarray(bias_full[idx].reshape(8, P).T),
                "c_t": np.ascontiguousarray(C_prev[:, c * S : (c + 1) * S].T),
            }
        )

    _CACHE["last_in_maps"] = in_maps
    res = run_bass_kernel_spmd(nc, in_maps, core_ids=list(range(NCORES)))

    def gather(name):
        t = np.concatenate([res.results[c][name] for c in range(NCORES)], axis=0)
        return np.ascontiguousarray(t.T)  # [B, H]

    return (
        gather("h_t"),
        gather("cn_t"),
        gather("f_t"),
        gather("i_t"),
        gather("g_t"),
        gather("o_t"),
    )


# revision 18
# speedup vs baseline: 1.1688x; 1.1373x over previous
"""TRN2 Bass kernel: fused LSTM cell (nn_CustomLSTMCell), 8-core tensor-parallel.

Strategy
--------
gates = x @ W_ih.T + b_ih + h_prev @ W_hh.T + b_hh  is computed as ONE GEMM
with contraction K = I + H = 4096 over xh = [x | h_prev] and W = [W_ih | W_hh].

The 4H gate dimension is tensor-parallel sharded across the 8 cores: core c
owns h-columns [c*256, (c+1)*256) of every gate (i, f, g, o).  Each core
computes gatesT [1024, 2048] = Wc @ xh.T with gate rows on partitions, so the
per-gate bias is a native per-partition scalar in scalar.activation, which
also applies sigmoid/tanh while evicting PSUM -> SBUF.  The LSTM cell update
(new_C = f*C + i*g, new_h = o*tanh(new_C)) runs on the vector engine, fully
overlapped with the tensor engine.  No collectives: output slices are
disjoint and gathered on the host.

Matmul operands are cast to fp16 on the host (halves DMA traffic, 4x PE rate
vs fp32, ~8x more mantissa than bf16); accumulation stays fp32 in PSUM and
the epilogue is fp32.

Schedule: hb (128-row half of the per-core gate slice) is the OUTER loop and
the batch tile n the inner loop, so only the lo half of W (4MB) is needed in
the front window.  The first TWO batch tiles are fused into one double-wide
group (chunk-major over both 512-col halves, 8 PSUM banks) so W-lo and the
first two xh tiles are consumed as per-chunk triples over 54.6us (~225GB/s,
inside the ~300GB/s the queues sustain).  Phase hi re-reads the xh tiles.

DMA dispatch instructions cost ~0.6us of engine sequencer time EACH, so the
dispatch count is the scarce resource (a per-chunk re-read schedule needed
~290 dispatches and starved the PE for 21us waiting on un-dispatched loads).
Only the front window uses per-chunk DMAs (pacing matters there); every
prefetched xh set, W-hi, and C ride ONE large DMA each -- they land a full
group ahead, so whole-set completion granularity costs nothing.

A short PE warm-up (6 dummy matmuls, ~2.6us) bridges the gap until chunk 0
lands and starts the HAM clock-gate warm-up early.  All output stores go on
the two HWDGE queues (sync/scalar); the final group pre-stores f/i/g/cn
during the o-gate matmul block so only o-sigmoid -> h-mul -> 2 half-stores
remain after the last MM.
"""

import numpy as np

B = 2048           # batch
I_DIM = 2048       # input features
H = 2048           # hidden
NCORES = 8
S = H // NCORES    # 256: per-core h-slice (per gate)
M_PER_CORE = 4 * S # 1024 gate rows per core
K = I_DIM + H      # 4096 fused contraction dim
P = 128
KC = K // P        # 32 contraction chunks
NT = B // 512      # 4 batch tiles of 512

_BF16 = np.float16

_CACHE = {}


def _build_program():
    from contextlib import ExitStack

    import concourse.mybir as mybir
    import concourse.tile as tile
    from concourse import bacc

    f32 = mybir.dt.float32
    bf16 = mybir.dt.float16
    AF = mybir.ActivationFunctionType

    nc = bacc.Bacc("TRN2", target_bir_lowering=False, debug=False)

    w_t = nc.dram_tensor("w_t", [K, M_PER_CORE], bf16, kind="ExternalInput").ap()
    xh_t = nc.dram_tensor("xh_t", [K, B], bf16, kind="ExternalInput").ap()
    # bias pre-shaped [128, 8] on the host: one clean 2D DMA (32B/partition)
    bias_d = nc.dram_tensor("bias", [P, 8], f32, kind="ExternalInput").ap()
    c_t = nc.dram_tensor("c_t", [S, B], f32, kind="ExternalInput").ap()
    outs = {
        name: nc.dram_tensor(name, [S, B], f32, kind="ExternalOutput").ap()
        for name in ("h_t", "cn_t", "f_t", "i_t", "g_t", "o_t")
    }

    # DRAM views with the 128-partition dim innermost on rows.
    w_r = w_t.rearrange("(a p) m -> p a m", p=P)        # [128, 32, 1024]
    xh_r = xh_t.rearrange("(a p) n -> p a n", p=P)      # [128, 32, 2048]
    c_r = c_t.rearrange("(a p) n -> p a n", p=P)        # [128, 2, 2048]

    HB = S // P  # 2 h-blocks of 128 per core
    # gate order within the per-core M dim: m-tile = hb*4 + gate (i,f,g,o)
    ACT_FN = [AF.Sigmoid, AF.Sigmoid, AF.Tanh, AF.Sigmoid]

    # group schedule: hb outer, n inner; groups 0+1 are fused (see docstring)
    SCHED = [(0, 0), (0, 1), (0, 2), (0, 3), (1, 0), (1, 1), (1, 2), (1, 3)]

    with tile.TileContext(nc) as tc, ExitStack() as ctx:
        w_pool = ctx.enter_context(tc.tile_pool(name="w", bufs=1))
        xh_pool = ctx.enter_context(tc.tile_pool(name="xh", bufs=3))
        c_pool = ctx.enter_context(tc.tile_pool(name="c", bufs=1))
        b_pool = ctx.enter_context(tc.tile_pool(name="b", bufs=1))
        psum_pool = ctx.enter_context(tc.tile_pool(name="ps", bufs=2, space="PSUM"))
        act_pool = ctx.enter_context(tc.tile_pool(name="act", bufs=2))

        in_eng = [nc.sync, nc.gpsimd]
        _in_rr = [0]

        def in_dma(dst, src):
            in_eng[_in_rr[0] % 2].dma_start(dst, src)
            _in_rr[0] += 1

        in3_eng = [nc.sync, nc.gpsimd, nc.scalar]
        _in3_rr = [0]

        def in3_dma(dst, src):
            in3_eng[_in3_rr[0] % 3].dma_start(dst, src)
            _in3_rr[0] += 1

        # Mid-kernel output stores ride the Scalar HWDGE queue, naturally
        # interleaved behind the activations that produce them.
        def out_dma(dst, src):
            nc.scalar.dma_start(dst, src)

        bias_all = b_pool.tile([P, 4 * HB], f32)

        # W-lo: per-chunk tiles -- 128KB DMAs produce 1KB-per-partition
        # packets, measured FASTER per byte than the 2KB packets that
        # chunk-pair DMAs generate.  W-hi: two 2MB halves (8KB+ packets,
        # also fast), emitted a group apart so each lands with slack
        # before phase hi consumes it.
        w_lo = [w_pool.tile([P, 4 * P], bf16, name=f"wl{k}") for k in range(KC)]
        w_hi = [
            w_pool.tile([P, KC // 2, 4 * P], bf16, name=f"wh{h}")
            for h in range(2)
        ]

        def w_slice(hb, k, g):
            if hb == 0:
                return w_lo[k][:, g * P : (g + 1) * P]
            return w_hi[k // 16][:, k % 16, g * P : (g + 1) * P]

        # All xh sets are uniform 3D tiles (same pool slot footprint).
        # Sets 0-2 (front window, pacing matters) are written by 32
        # per-chunk DMAs into [:, k, :] slices -- Tile tracks sub-tile
        # ranges, so each matmul waits only on its own chunk's DMA.
        # Sets 3-7 are prefetched a full group ahead with ONE 4MB DMA each
        # (dispatch instructions cost ~0.6us of engine time apiece).
        def alloc_xh():
            return xh_pool.tile([P, KC, 512], bf16, name="xhs")

        xh_sets = {0: alloc_xh(), 1: alloc_xh(), 2: alloc_xh()}

        def xh_chunk(i, k):
            return xh_sets[i][:, k, :]

        # Preamble.  Chunk 0 of the (w, xh0, xh1) triple goes out first on
        # the HWDGE queues so the first matmul can start ASAP; the rest
        # interleaves per-chunk triples over all three queues in the fused
        # group's consumption order, then C-lo, then set 2 (consumed
        # chunk-paced by group 2 right after the fused group).
        # Queue capacities under 3-way HBM contention: sync ~ scalar ~
        # 115GB/s, gpsimd (SWDGE) ~65GB/s.  Every stream is split so each
        # queue's FIFO meets its deadline: fused window (13MB/58us) puts
        # w on sync, xh0 on scalar, xh1 split 8/8/16; set 2 (group-2
        # deadline) splits 8/8/16 the other way.
        nc.sync.dma_start(w_lo[0][:], w_r[:, 0, 0 : 4 * P])
        nc.sync.dma_start(xh_sets[0][:, 0, :], xh_r[:, 0, 0:512])
        nc.scalar.dma_start(xh_sets[1][:, 0, :], xh_r[:, 0, 512:1024])
        # bias is tiny and first needed at ~65us; keep it off chunk 0's path
        nc.sync.dma_start(bias_all[:], bias_d[:, :])
        for k in range(1, KC):
            nc.sync.dma_start(w_lo[k][:], w_r[:, k, 0 : 4 * P])
            nc.scalar.dma_start(xh_sets[0][:, k, :], xh_r[:, k, 0:512])
            if k % 2 == 0:
                eng = nc.gpsimd
            else:
                eng = nc.sync if k % 4 == 1 else nc.scalar
            eng.dma_start(xh_sets[1][:, k, :], xh_r[:, k, 512:1024])
        c_tiles = {0: c_pool.tile([P, B], f32, name="c")}
        nc.scalar.dma_start(c_tiles[0][:], c_r[:, 0, :])
        for k in range(KC):
            if k % 2 == 0:
                eng = nc.gpsimd
            else:
                eng = nc.sync if k % 4 == 1 else nc.scalar
            eng.dma_start(xh_sets[2][:, k, :], xh_r[:, k, 1024:1536])

        # PE warm-up: a few matmuls on dummy data bridge the gap until
        # chunk 0 lands (~10.5us) and start the HAM warm-up window early.
        dummy = b_pool.tile([P, 512], bf16)
        nc.vector.memset(dummy[:], 0.0)
        warm_ps = psum_pool.tile([P, 512], f32, name="ps0")
        NWARM = 8
        for i in range(NWARM):
            nc.tensor.matmul(
                warm_ps[:], dummy[:, 0:P], dummy[:],
                start=(i == 0), stop=(i == NWARM - 1),
            )

        def epilogue(hb, ns, ps, final):
            # activations (+bias) evict PSUM, then the cell update.  For
            # the final group: f,i,g evict first and the cell-state chain
            # plus the f/i/g/cn stores complete during the o matmul block.
            def gate_act(g):
                m = hb * 4 + g
                a = act_pool.tile([P, 512], f32, name=f"a{g}")
                nc.scalar.activation(
                    a[:], ps[g][:], ACT_FN[g], bias=bias_all[:, m : m + 1]
                )
                return a

            acts = [None] * 4
            for g in (1, 0, 2) if final else (0, 1, 2, 3):
                acts[g] = gate_act(g)
            fc = act_pool.tile([P, 512], f32, name="fc")
            nc.vector.tensor_mul(fc[:], acts[1][:], c_tiles[hb][:, ns])
            ig = act_pool.tile([P, 512], f32, name="ig")
            nc.vector.tensor_mul(ig[:], acts[0][:], acts[2][:])
            cn = act_pool.tile([P, 512], f32, name="cn")
            nc.vector.tensor_add(cn[:], ig[:], fc[:])
            th = act_pool.tile([P, 512], f32, name="th")
            nc.scalar.activation(th[:], cn[:], AF.Tanh)

            rs = slice(hb * P, (hb + 1) * P)
            if final:
                # pre-store everything ready while o's matmuls run
                nc.sync.dma_start(outs["f_t"][rs, ns], acts[1][:])
                nc.sync.dma_start(outs["i_t"][rs, ns], acts[0][:])
                nc.scalar.dma_start(outs["g_t"][rs, ns], acts[2][:])
                nc.sync.dma_start(outs["cn_t"][rs, ns], cn[:])
                acts[3] = gate_act(3)
            hn = act_pool.tile([P, 512], f32, name="hn")
            nc.vector.tensor_mul(hn[:], acts[3][:], th[:])

            if final:
                # split h/o across both HWDGE queues for parallel receipt
                lo = slice(ns.start, ns.start + 256)
                hi = slice(ns.start + 256, ns.stop)
                nc.scalar.dma_start(outs["o_t"][rs, lo], acts[3][:, 0:256])
                nc.sync.dma_start(outs["o_t"][rs, hi], acts[3][:, 256:512])
                nc.sync.dma_start(outs["h_t"][rs, lo], hn[:, 0:256])
                nc.scalar.dma_start(outs["h_t"][rs, hi], hn[:, 256:512])
            else:
                pairs = (
                    ("f_t", acts[1]),
                    ("i_t", acts[0]),
                    ("g_t", acts[2]),
                    ("o_t", acts[3]),
                    ("cn_t", cn),
                    ("h_t", hn),
                )
                for name, t in pairs:
                    out_dma(outs[name][rs, ns], t[:])

        # ---- fused double-wide first group: batch cols [0:1024], hb=0 ----
        # chunk-major over both halves: the (w, xh0, xh1) triple of chunk k
        # is consumed at ~k*1.7us, matching the preamble queue order.
        ps01 = [
            [psum_pool.tile([P, 512], f32, name=f"ps{g}") for g in range(4)]
            for _ in range(2)
        ]
        for k in range(KC):
            for half in range(2):
                for g in range(4):
                    nc.tensor.matmul(
                        ps01[half][g][:],
                        w_slice(0, k, g),
                        xh_chunk(half, k),
                        start=(k == 0),
                        stop=(k == KC - 1),
                    )
        # set 3: pool slot is set 0's, so it waits for the fused group's
        # reads and lands during group 2, consumed in group 3.  Two 2MB
        # halves so sync and scalar each carry one.
        xh_sets[3] = alloc_xh()
        nc.sync.dma_start(xh_sets[3][:, 0:16, :], xh_r[:, 0:16, 1536:2048])
        nc.scalar.dma_start(xh_sets[3][:, 16:32, :], xh_r[:, 16:32, 1536:2048])
        # W-hi half 0 after the fused group: gpsimd queues it behind
        # nothing deadline-critical; lands during group 2/3.
        for half in range(2):
            epilogue(0, slice(half * 512, (half + 1) * 512), ps01[half], False)

        # ---- groups 2..7 ----
        for i in range(2, len(SCHED)):
            final = i == len(SCHED) - 1
            hb, n = SCHED[i]
            ns = slice(n * 512, (n + 1) * 512)

            ps = [
                psum_pool.tile([P, 512], f32, name=f"ps{g}") for g in range(4)
            ]
            if final:
                # gate-major (f,i,g,o): each gate's PSUM closes early so the
                # epilogue chain after the very last matmul is just
                # o-sigmoid -> h-mul -> store.
                for g in (1, 0, 2, 3):
                    for k in range(KC):
                        nc.tensor.matmul(
                            ps[g][:],
                            w_slice(hb, k, g),
                            xh_chunk(i, k),
                            start=(k == 0),
                            stop=(k == KC - 1),
                        )
            else:
                for k in range(KC):
                    for g in range(4):
                        nc.tensor.matmul(
                            ps[g][:],
                            w_slice(hb, k, g),
                            xh_chunk(i, k),
                            start=(k == 0),
                            stop=(k == KC - 1),
                        )

            # Prefetch xh set i+2 as ONE DMA after this group's matmuls:
            # its pool-slot wait self-paces it into the next group's window.
            if i + 2 < len(SCHED):
                xh_sets[i + 2] = alloc_xh()
                nn = SCHED[i + 2][1]
                cs = slice(nn * 512, (nn + 1) * 512)
                nc.sync.dma_start(xh_sets[i + 2][:, 0:16, :], xh_r[:, 0:16, cs])
                nc.scalar.dma_start(xh_sets[i + 2][:, 16:32, :], xh_r[:, 16:32, cs])
            # W-hi halves after groups 2 and 3 (each lands a group before
            # phase hi needs it); C-hi (slot of C-lo) waits for group 3's
            # epilogue and lands mid-group-4, before its epilogue needs it.
            if i == 2:
                nc.sync.dma_start(w_hi[0][:], w_r[:, 0:16, 4 * P : 8 * P])
                nc.scalar.dma_start(w_hi[1][:], w_r[:, 16:32, 4 * P : 8 * P])
            elif i == 3:
                c_tiles[1] = c_pool.tile([P, B], f32, name="c")
                nc.gpsimd.dma_start(c_tiles[1][:], c_r[:, 1, :])

            epilogue(hb, ns, ps, final)

    nc.compile()
    return nc


def _get_program():
    if "nc" not in _CACHE:
        _CACHE["nc"] = _build_program()
    return _CACHE["nc"]


def _gate_row_index(core: int) -> np.ndarray:
    """Global rows of W/b (4H-dim) owned by `core`, in m-tile order."""
    idx = []
    for hb in range(S // P):
        for g in range(4):
            base = g * H + core * S + hb * P
            idx.extend(range(base, base + P))
    return np.asarray(idx)


def kernel(x, h_prev, C_prev, W_ih, b_ih, W_hh, b_hh):
    from concourse.bass_utils import run_bass_kernel_spmd

    nc = _get_program()

    xh_t = np.ascontiguousarray(
        np.concatenate([x, h_prev], axis=1).T
    ).astype(_BF16)  # [4096, 2048], shared by all cores
    bias_full = (b_ih + b_hh).astype(np.float32)

    in_maps = []
    for c in range(NCORES):
        idx = _gate_row_index(c)
        w_cat = np.concatenate([W_ih[idx], W_hh[idx]], axis=1)  # [1024, 4096]
        in_maps.append(
            {
                "w_t": np.ascontiguousarray(w_cat.T).astype(_BF16),
                "xh_t": xh_t,
                "bias": np.ascontiguousarray(bias_full[idx].reshape(8, P).T),
                "c_t": np.ascontiguousarray(C_prev[:, c * S : (c + 1) * S].T),
            }
        )

    _CACHE["last_in_maps"] = in_maps
    res = run_bass_kernel_spmd(nc, in_maps, core_ids=list(range(NCORES)))

    def gather(name):
        t = np.concatenate([res.results[c][name] for c in range(NCORES)], axis=0)
        return np.ascontiguousarray(t.T)  # [B, H]

    return (
        gather("h_t"),
        gather("cn_t"),
        gather("f_t"),
        gather("i_t"),
        gather("g_t"),
        gather("o_t"),
    )


# revision 19
# speedup vs baseline: 1.2012x; 1.0278x over previous
"""TRN2 Bass kernel: fused LSTM cell (nn_CustomLSTMCell), 8-core tensor-parallel.

Strategy
--------
gates = x @ W_ih.T + b_ih + h_prev @ W_hh.T + b_hh  is computed as ONE GEMM
with contraction K = I + H = 4096 over xh = [x | h_prev] and W = [W_ih | W_hh].

The 4H gate dimension is tensor-parallel sharded across the 8 cores: core c
owns h-columns [c*256, (c+1)*256) of every gate (i, f, g, o).  Each core
computes gatesT [1024, 2048] = Wc @ xh.T with gate rows on partitions, so the
per-gate bias is a native per-partition scalar in scalar.activation, which
also applies sigmoid/tanh while evicting PSUM -> SBUF.  The LSTM cell update
(new_C = f*C + i*g, new_h = o*tanh(new_C)) runs on the vector engine, fully
overlapped with the tensor engine.  No collectives: output slices are
disjoint and gathered on the host.

Matmul operands are cast to fp16 on the host (halves DMA traffic, 4x PE rate
vs fp32, ~8x more mantissa than bf16); accumulation stays fp32 in PSUM and
the epilogue is fp32.

Schedule: hb (128-row half of the per-core gate slice) is the OUTER loop and
the batch tile n the inner loop, so only the lo half of W (4MB) is needed in
the front window.  The first TWO batch tiles are fused into one double-wide
group (chunk-major over both 512-col halves, 8 PSUM banks) so W-lo and the
first two xh tiles are consumed as per-chunk triples over 54.6us (~225GB/s,
inside the ~300GB/s the queues sustain).  Phase hi re-reads the xh tiles.

DMA dispatch instructions cost ~0.6us of engine sequencer time EACH, so the
dispatch count is the scarce resource (a per-chunk re-read schedule needed
~290 dispatches and starved the PE for 21us waiting on un-dispatched loads).
Only the front window uses per-chunk DMAs (pacing matters there); every
prefetched xh set, W-hi, and C ride ONE large DMA each -- they land a full
group ahead, so whole-set completion granularity costs nothing.

A short PE warm-up (6 dummy matmuls, ~2.6us) bridges the gap until chunk 0
lands and starts the HAM clock-gate warm-up early.  All output stores go on
the two HWDGE queues (sync/scalar); the final group pre-stores f/i/g/cn
during the o-gate matmul block so only o-sigmoid -> h-mul -> 2 half-stores
remain after the last MM.
"""

import numpy as np

B = 2048           # batch
I_DIM = 2048       # input features
H = 2048           # hidden
NCORES = 8
S = H // NCORES    # 256: per-core h-slice (per gate)
M_PER_CORE = 4 * S # 1024 gate rows per core
K = I_DIM + H      # 4096 fused contraction dim
P = 128
KC = K // P        # 32 contraction chunks
NT = B // 512      # 4 batch tiles of 512

_BF16 = np.float16

_CACHE = {}


def _build_program():
    from contextlib import ExitStack

    import concourse.mybir as mybir
    import concourse.tile as tile
    from concourse import bacc

    f32 = mybir.dt.float32
    bf16 = mybir.dt.float16
    AF = mybir.ActivationFunctionType

    nc = bacc.Bacc("TRN2", target_bir_lowering=False, debug=False)

    w_t = nc.dram_tensor("w_t", [K, M_PER_CORE], bf16, kind="ExternalInput").ap()
    xh_t = nc.dram_tensor("xh_t", [K, B], bf16, kind="ExternalInput").ap()
    # bias pre-shaped [128, 8] on the host: one clean 2D DMA (32B/partition)
    bias_d = nc.dram_tensor("bias", [P, 8], f32, kind="ExternalInput").ap()
    c_t = nc.dram_tensor("c_t", [S, B], f32, kind="ExternalInput").ap()
    outs = {
        name: nc.dram_tensor(name, [S, B], f32, kind="ExternalOutput").ap()
        for name in ("h_t", "cn_t", "f_t", "i_t", "g_t", "o_t")
    }

    # DRAM views with the 128-partition dim innermost on rows.
    w_r = w_t.rearrange("(a p) m -> p a m", p=P)        # [128, 32, 1024]
    xh_r = xh_t.rearrange("(a p) n -> p a n", p=P)      # [128, 32, 2048]
    c_r = c_t.rearrange("(a p) n -> p a n", p=P)        # [128, 2, 2048]

    HB = S // P  # 2 h-blocks of 128 per core
    # gate order within the per-core M dim: m-tile = hb*4 + gate (i,f,g,o)
    ACT_FN = [AF.Sigmoid, AF.Sigmoid, AF.Tanh, AF.Sigmoid]

    # group schedule: hb outer, n inner; groups 0+1 are fused (see docstring)
    SCHED = [(0, 0), (0, 1), (0, 2), (0, 3), (1, 0), (1, 1), (1, 2), (1, 3)]

    with tile.TileContext(nc) as tc, ExitStack() as ctx:
        w_pool = ctx.enter_context(tc.tile_pool(name="w", bufs=1))
        xh_pool = ctx.enter_context(tc.tile_pool(name="xh", bufs=3))
        c_pool = ctx.enter_context(tc.tile_pool(name="c", bufs=1))
        b_pool = ctx.enter_context(tc.tile_pool(name="b", bufs=1))
        psum_pool = ctx.enter_context(tc.tile_pool(name="ps", bufs=2, space="PSUM"))
        act_pool = ctx.enter_context(tc.tile_pool(name="act", bufs=2))

        in_eng = [nc.sync, nc.gpsimd]
        _in_rr = [0]

        def in_dma(dst, src):
            in_eng[_in_rr[0] % 2].dma_start(dst, src)
            _in_rr[0] += 1

        in3_eng = [nc.sync, nc.gpsimd, nc.scalar]
        _in3_rr = [0]

        def in3_dma(dst, src):
            in3_eng[_in3_rr[0] % 3].dma_start(dst, src)
            _in3_rr[0] += 1

        # Mid-kernel output stores ride the Scalar HWDGE queue, naturally
        # interleaved behind the activations that produce them.
        def out_dma(dst, src):
            nc.scalar.dma_start(dst, src)

        bias_all = b_pool.tile([P, 4 * HB], f32)

        # W-lo: per-chunk tiles -- 128KB DMAs produce 1KB-per-partition
        # packets, measured FASTER per byte than the 2KB packets that
        # chunk-pair DMAs generate.  W-hi: two 2MB halves (8KB+ packets,
        # also fast), emitted a group apart so each lands with slack
        # before phase hi consumes it.
        w_lo = [w_pool.tile([P, 4 * P], bf16, name=f"wl{k}") for k in range(KC)]
        w_hi = [
            w_pool.tile([P, KC // 2, 4 * P], bf16, name=f"wh{h}")
            for h in range(2)
        ]

        def w_slice(hb, k, g):
            if hb == 0:
                return w_lo[k][:, g * P : (g + 1) * P]
            return w_hi[k // 16][:, k % 16, g * P : (g + 1) * P]

        # All xh sets are uniform 3D tiles (same pool slot footprint).
        # Sets 0-2 (front window, pacing matters) are written by 32
        # per-chunk DMAs into [:, k, :] slices -- Tile tracks sub-tile
        # ranges, so each matmul waits only on its own chunk's DMA.
        # Sets 3-7 are prefetched a full group ahead with ONE 4MB DMA each
        # (dispatch instructions cost ~0.6us of engine time apiece).
        def alloc_xh():
            return xh_pool.tile([P, KC, 512], bf16, name="xhs")

        xh_sets = {0: alloc_xh(), 1: alloc_xh(), 2: alloc_xh()}

        def xh_chunk(i, k):
            return xh_sets[i][:, k, :]

        # Preamble.  Chunk 0 of the (w, xh0, xh1) triple goes out first on
        # the HWDGE queues so the first matmul can start ASAP; the rest
        # interleaves per-chunk triples over all three queues in the fused
        # group's consumption order, then C-lo, then set 2 (consumed
        # chunk-paced by group 2 right after the fused group).
        # Queue capacities under 3-way HBM contention: sync ~ scalar ~
        # 115GB/s, gpsimd (SWDGE) ~65GB/s.  Every stream is split so each
        # queue's FIFO meets its deadline: fused window (13MB/58us) puts
        # w on sync, xh0 on scalar, xh1 split 8/8/16; set 2 (group-2
        # deadline) splits 8/8/16 the other way.
        nc.sync.dma_start(w_lo[0][:], w_r[:, 0, 0 : 4 * P])
        nc.sync.dma_start(xh_sets[0][:, 0, :], xh_r[:, 0, 0:512])
        nc.scalar.dma_start(xh_sets[1][:, 0, :], xh_r[:, 0, 512:1024])
        # bias is tiny and first needed at ~65us; keep it off chunk 0's path
        nc.sync.dma_start(bias_all[:], bias_d[:, :])
        for k in range(1, KC):
            nc.sync.dma_start(w_lo[k][:], w_r[:, k, 0 : 4 * P])
            nc.scalar.dma_start(xh_sets[0][:, k, :], xh_r[:, k, 0:512])
            if k % 2 == 0:
                eng = nc.gpsimd
            else:
                eng = nc.sync if k % 4 == 1 else nc.scalar
            eng.dma_start(xh_sets[1][:, k, :], xh_r[:, k, 512:1024])
        c_tiles = {0: c_pool.tile([P, B], f32, name="c")}
        nc.scalar.dma_start(c_tiles[0][:], c_r[:, 0, :])
        # set 2's first chunks load early so the group-2 start never waits
        for k in range(0, 3):
            nc.sync.dma_start(xh_sets[2][:, k, :], xh_r[:, k, 1024:1536])
        for k in range(3, KC):
            if k % 2 == 0:
                eng = nc.gpsimd
            else:
                eng = nc.sync if k % 4 == 1 else nc.scalar
            eng.dma_start(xh_sets[2][:, k, :], xh_r[:, k, 1024:1536])

        # PE warm-up: a few matmuls on dummy data bridge the gap until
        # chunk 0 lands (~10.5us) and start the HAM warm-up window early.
        dummy = b_pool.tile([P, 512], bf16)
        nc.vector.memset(dummy[:], 0.0)
        warm_ps = psum_pool.tile([P, 512], f32, name="ps0")
        NWARM = 12
        for i in range(NWARM):
            nc.tensor.matmul(
                warm_ps[:], dummy[:, 0:P], dummy[:],
                start=(i == 0), stop=(i == NWARM - 1),
            )

        def epilogue(hb, ns, ps, final):
            # activations (+bias) evict PSUM, then the cell update.  For
            # the final group: f,i,g evict first and the cell-state chain
            # plus the f/i/g/cn stores complete during the o matmul block.
            def gate_act(g):
                m = hb * 4 + g
                a = act_pool.tile([P, 512], f32, name=f"a{g}")
                nc.scalar.activation(
                    a[:], ps[g][:], ACT_FN[g], bias=bias_all[:, m : m + 1]
                )
                return a

            acts = [None] * 4
            for g in (1, 0, 2) if final else (0, 1, 2, 3):
                acts[g] = gate_act(g)
            fc = act_pool.tile([P, 512], f32, name="fc")
            nc.vector.tensor_mul(fc[:], acts[1][:], c_tiles[hb][:, ns])
            ig = act_pool.tile([P, 512], f32, name="ig")
            nc.vector.tensor_mul(ig[:], acts[0][:], acts[2][:])
            cn = act_pool.tile([P, 512], f32, name="cn")
            nc.vector.tensor_add(cn[:], ig[:], fc[:])
            th = act_pool.tile([P, 512], f32, name="th")
            nc.scalar.activation(th[:], cn[:], AF.Tanh)

            rs = slice(hb * P, (hb + 1) * P)
            if final:
                # pre-store everything ready while o's matmuls run
                nc.sync.dma_start(outs["f_t"][rs, ns], acts[1][:])
                nc.sync.dma_start(outs["i_t"][rs, ns], acts[0][:])
                nc.scalar.dma_start(outs["g_t"][rs, ns], acts[2][:])
                nc.sync.dma_start(outs["cn_t"][rs, ns], cn[:])
                acts[3] = gate_act(3)
            hn = act_pool.tile([P, 512], f32, name="hn")
            nc.vector.tensor_mul(hn[:], acts[3][:], th[:])

            if final:
                # split h/o across both HWDGE queues for parallel receipt
                lo = slice(ns.start, ns.start + 256)
                hi = slice(ns.start + 256, ns.stop)
                nc.scalar.dma_start(outs["o_t"][rs, lo], acts[3][:, 0:256])
                nc.sync.dma_start(outs["o_t"][rs, hi], acts[3][:, 256:512])
                nc.sync.dma_start(outs["h_t"][rs, lo], hn[:, 0:256])
                nc.scalar.dma_start(outs["h_t"][rs, hi], hn[:, 256:512])
            else:
                pairs = (
                    ("f_t", acts[1]),
                    ("i_t", acts[0]),
                    ("g_t", acts[2]),
                    ("o_t", acts[3]),
                    ("cn_t", cn),
                    ("h_t", hn),
                )
                for name, t in pairs:
                    out_dma(outs[name][rs, ns], t[:])

        # ---- fused double-wide first group: batch cols [0:1024], hb=0 ----
        # chunk-major over both halves: the (w, xh0, xh1) triple of chunk k
        # is consumed at ~k*1.7us, matching the preamble queue order.
        ps01 = [
            [psum_pool.tile([P, 512], f32, name=f"ps{g}") for g in range(4)]
            for _ in range(2)
        ]
        # Last 4 chunks run half-0 first, then half-1: half-0's PSUM banks
        # close ~3.5us before the stream ends, so their evictions overlap
        # the half-1 tail and group 2's first matmuls start almost
        # immediately after the fused group.
        fused_order = [(k, h) for k in range(KC - 4) for h in range(2)]
        fused_order += [(k, 0) for k in range(KC - 4, KC)]
        fused_order += [(k, 1) for k in range(KC - 4, KC)]
        for k, half in fused_order:
            for g in range(4):
                nc.tensor.matmul(
                    ps01[half][g][:],
                    w_slice(0, k, g),
                    xh_chunk(half, k),
                    start=(k == 0),
                    stop=(k == KC - 1),
                )
        # set 3: pool slot is set 0's, so it waits for the fused group's
        # reads and lands during group 2, consumed in group 3.  Two 2MB
        # halves so sync and scalar each carry one.
        xh_sets[3] = alloc_xh()
        nc.sync.dma_start(xh_sets[3][:, 0:16, :], xh_r[:, 0:16, 1536:2048])
        nc.scalar.dma_start(xh_sets[3][:, 16:32, :], xh_r[:, 16:32, 1536:2048])
        # W-hi half 0 after the fused group: gpsimd queues it behind
        # nothing deadline-critical; lands during group 2/3.
        for half in range(2):
            epilogue(0, slice(half * 512, (half + 1) * 512), ps01[half], False)

        # ---- groups 2..7 ----
        for i in range(2, len(SCHED)):
            final = i == len(SCHED) - 1
            hb, n = SCHED[i]
            ns = slice(n * 512, (n + 1) * 512)

            ps = [
                psum_pool.tile([P, 512], f32, name=f"ps{g}") for g in range(4)
            ]
            if final:
                # gate-major (f,i,g,o): each gate's PSUM closes early so the
                # epilogue chain after the very last matmul is just
                # o-sigmoid -> h-mul -> store.
                for g in (1, 0, 2, 3):
                    for k in range(KC):
                        nc.tensor.matmul(
                            ps[g][:],
                            w_slice(hb, k, g),
                            xh_chunk(i, k),
                            start=(k == 0),
                            stop=(k == KC - 1),
                        )
            else:
                for k in range(KC):
                    for g in range(4):
                        nc.tensor.matmul(
                            ps[g][:],
                            w_slice(hb, k, g),
                            xh_chunk(i, k),
                            start=(k == 0),
                            stop=(k == KC - 1),
                        )

            # Prefetch xh set i+2 as ONE DMA after this group's matmuls:
            # its pool-slot wait self-paces it into the next group's window.
            if i + 2 < len(SCHED):
                xh_sets[i + 2] = alloc_xh()
                nn = SCHED[i + 2][1]
                cs = slice(nn * 512, (nn + 1) * 512)
                nc.sync.dma_start(xh_sets[i + 2][:, 0:16, :], xh_r[:, 0:16, cs])
                nc.scalar.dma_start(xh_sets[i + 2][:, 16:32, :], xh_r[:, 16:32, cs])
            # W-hi halves after groups 2 and 3 (each lands a group before
            # phase hi needs it); C-hi (slot of C-lo) waits for group 3's
            # epilogue and lands mid-group-4, before its epilogue needs it.
            if i == 2:
                nc.sync.dma_start(w_hi[0][:], w_r[:, 0:16, 4 * P : 8 * P])
                nc.scalar.dma_start(w_hi[1][:], w_r[:, 16:32, 4 * P : 8 * P])
            elif i == 3:
                c_tiles[1] = c_pool.tile([P, B], f32, name="c")
                nc.gpsimd.dma_start(c_tiles[1][:], c_r[:, 1, :])

            epilogue(hb, ns, ps, final)

    nc.compile()
    return nc


def _get_program():
    if "nc" not in _CACHE:
        _CACHE["nc"] = _build_program()
    return _CACHE["nc"]


def _gate_row_index(core: int) -> np.ndarray:
    """Global rows of W/b (4H-dim) owned by `core`, in m-tile order."""
    idx = []
    for hb in range(S // P):
        for g in range(4):
            base = g * H + core * S + hb * P
            idx.extend(range(base, base + P))
    return np.asarray(idx)


def kernel(x, h_prev, C_prev, W_ih, b_ih, W_hh, b_hh):
    from concourse.bass_utils import run_bass_kernel_spmd

    nc = _get_program()

    xh_t = np.ascontiguousarray(
        np.concatenate([x, h_prev], axis=1).T
    ).astype(_BF16)  # [4096, 2048], shared by all cores
    bias_full = (b_ih + b_hh).astype(np.float32)

    in_maps = []
    for c in range(NCORES):
        idx = _gate_row_index(c)
        w_cat = np.concatenate([W_ih[idx], W_hh[idx]], axis=1)  # [1024, 4096]
        in_maps.append(
            {
                "w_t": np.ascontiguousarray(w_cat.T).astype(_BF16),
                "xh_t": xh_t,
                "bias": np.ascontiguousarray(bias_full[idx].reshape(8, P).T),
                "c_t": np.ascontiguousarray(C_prev[:, c * S : (c + 1) * S].T),
            }
        )

    _CACHE["last_in_maps"] = in_maps
    res = run_bass_kernel_spmd(nc, in_maps, core_ids=list(range(NCORES)))

    def gather(name):
        t = np.concatenate([res.results[c][name] for c in range(NCORES)], axis=0)
        return np.ascontiguousarray(t.T)  # [B, H]

    return (
        gather("h_t"),
        gather("cn_t"),
        gather("f_t"),
        gather("i_t"),
        gather("g_t"),
        gather("o_t"),
    )
